# revision 4
# baseline (speedup 1.0000x reference)
"""Batched DWI 3D conv as implicit GEMM on 8 TRN2 NeuronCores.

Problem: x [8, 12, 12, 32, 32, 32] f32, W [32, 12, 12, 3, 3, 3] f32
         -> out [8, 32, 30, 30, 30] f32  (VALID 3D conv, c_in = 144)

Strategy (data-parallel over batch, one batch element per core):
  - x viewed as [144, 32, 32, 32] bf16 in SBUF; a kernel offset (dx, dy, dz)
    is a pure free-dim shift, so the conv is a chain of shifted matmuls
    accumulated in PSUM: out[co, n] += W_d^T @ x[:, n + shift(d)].
  - c_out = 32 fills only 1/4 of the PE array columns, so 4 col-tiled
    matmuls run concurrently (tile_position=(0, 32j)), each computing a
    different output chunk into its own 32-partition PSUM slice. Measured:
    a 4-matmul phase streams in ~190 ns = one N=450 pass at the warm
    2.4 GHz PE clock, LDWEIGHTS hidden.
  - c_in = 144 = 128 + 16. The 128-channel body: 27 passes (one per tap),
    K=128, shifts via AP offsets (plane index = dx, row = dy, col = dz).
  - The 16-channel tail is packed as a host-built [128, .] tile of 8
    shifted copies (shifts SHIFTS below). 5 tail passes with AP base
    offsets BASES cover all 27 taps exactly once (verified cover; the
    13 surplus (pass, block) slots carry zero weights). 32 passes per
    chunk total vs the naive 27 + 6.
  - Per output chunk (fixed ox, 15 y-rows, 30 z): 32 matmul phases
    accumulate into one PSUM-bank slice, N = 450.
  - DMA pieces are issued in deadline order (each piece lands just before
    the phase that first consumes it); issue cost on the sync queue is
    ~0.65 us per DMA_DIRECT2D, so the first x piece is small to get the
    PE started early. Outputs are stored as bf16 (cast back on host) to
    halve store traffic. Dummy matmuls bridge the PE from the framework
    preamble to the first real phase so the HAM clock gate (4/8 -> 8/8
    after ~3.4 us of sustained activity) is warming during the load.
  - All DMAs stay on the sync HWDGE ring: moving any traffic to the ACT
    HWDGE ring or gpsimd SWDGE measurably degrades aggregate bandwidth.
  Stall budget (from perfetto): ~6.8 us framework preamble, first real
  matmul ~10 us, 480 phases x ~190 ns, one known-unexplained ~4 us
  tensor-engine freeze mid-run (firmware/power; survives scheduling).
"""

import numpy as np
import ml_dtypes

import concourse.bass as bass
import concourse.bacc as bacc
import concourse.mybir as mybir
import concourse.tile as tile
from concourse.bass_utils import run_bass_kernel_spmd

BF16 = mybir.dt.bfloat16
F32 = mybir.dt.float32

N_CORES = 8
CIN = 144
COUT = 32
DIM = 32
ODIM = 30
NCH = 450  # one chunk = 15 y-rows x 30 z
NFLAT = DIM * DIM * DIM
CHUNKS = [(ox, h) for ox in range(ODIM) for h in (0, 1)]  # 60 chunks
# chunks per group (each group = nbank psum banks x 4 col tiles); group sizes
# ramp up so the PE starts after only the first small x slab lands
GROUP_SIZES = [4, 8, 16, 16, 12, 4]
# x body loaded in plane slabs sized to stay ahead of PE consumption
XSLABS = [(0, 2), (2, 4), (4, 8), (8, 16), (16, 24), (24, 32)]
# tail: 8 pre-shifted copies of the 16 tail channels; 5 passes with AP base
# offsets cover all 27 taps exactly once (host asserts the cover)
T_ROWS = 128
TAIL_SHIFTS = [0, 32, 64, 1024, 1057, 1058, 2048, 2112]
TAIL_BASES = [(0, 0, 0), (0, 0, 1), (0, 0, 2), (0, 1, 0), (1, 0, 0)]
TPIECES = [(0, 3), (3, 8), (8, 16), (16, 32)]
N_TAIL = len(TAIL_BASES)
WARM_N = 128
WARM_CNT = 24

_CACHE = {}


def _ctiles():
    out = []
    for dx in range(3):
        for dy in range(3):
            for dz in range(3):
                out.append(("full", dx, dy, dz))
    for k, (da, db, dc) in enumerate(TAIL_BASES):
        out.append(("tail", k, 0, 0))
    return out


def build_nc():
    nc = bacc.Bacc(None, target_bir_lowering=False)
    xin = nc.dram_tensor("x", [128, DIM, DIM, DIM], BF16, kind="ExternalInput")
    xt_d = nc.dram_tensor("xt", [T_ROWS, DIM, DIM, DIM], BF16, kind="ExternalInput")
    wf_d = nc.dram_tensor("wf", [128, 27, COUT], BF16, kind="ExternalInput")
    wt_d = nc.dram_tensor("wt", [T_ROWS, N_TAIL, COUT], BF16, kind="ExternalInput")
    # output laid out [partition = 32*colgroup + co, bank_seq, 450] so each
    # group's store is one DMA with contiguous per-partition records; bf16
    # to halve store traffic (host casts back to f32)
    n_banks_total = sum(g // 4 for g in GROUP_SIZES)
    out_d = nc.dram_tensor("out", [128, n_banks_total, NCH], BF16, kind="ExternalOutput")

    ctiles = _ctiles()
    last = len(ctiles) - 1

    with tile.TileContext(nc) as tc:
        with (
            tc.tile_pool(name="wpool", bufs=1) as wpool,
            tc.tile_pool(name="xpool", bufs=1) as xpool,
            tc.tile_pool(name="tpool", bufs=1) as tpool,
            tc.tile_pool(name="spool", bufs=3) as spool,
            tc.tile_pool(name="ppool", bufs=8, space="PSUM") as ppool,
        ):
            WF = wpool.tile([128, 27, COUT], BF16, tag="wf")
            WT = wpool.tile([T_ROWS, N_TAIL, COUT], BF16, tag="wt")

            XPG = []
            for si, (p0, p1) in enumerate(XSLABS):
                t = xpool.tile([128, p1 - p0, DIM, DIM], BF16, tag=f"xp{si}")
                XPG.append(t)
            T = tpool.tile([T_ROWS, DIM, DIM, DIM], BF16, tag="tail")

            def load_slab(si):
                p0, p1 = XSLABS[si]
                nc.sync.dma_start(XPG[si][:], xin[:, p0:p1, :, :])

            def load_tailp(pi):
                a, b = TPIECES[pi]
                nc.sync.dma_start(T[:, a:b, :, :], xt_d[:, a:b, :, :])

            # issue order = deadline order: each piece lands just ahead of
            # the phase that first consumes it; the first pieces are tiny
            # (ctile-0 weight column, 2 x planes) so the PE starts early
            nc.sync.dma_start(WF[:, 0:1, :], wf_d[:, 0:1, :])  # ctile-0 col
            load_slab(0)                       # planes 0-1
            load_slab(1)                       # planes 2-3
            nc.sync.dma_start(WF[:, 1:27, :], wf_d[:, 1:27, :])
            load_tailp(0)                      # tail planes 0-2
            load_slab(2)                       # planes 4-7
            nc.sync.dma_start(WT[:], wt_d[:])  # tail weights
            load_tailp(1)                      # tail planes 3-7
            load_slab(3)                       # planes 8-15
            load_tailp(2)                      # tail planes 8-15
            load_slab(4)                       # planes 16-23
            load_slab(5)                       # planes 24-31
            load_tailp(3)                      # tail planes 16-31

            # bridge the PE from the framework preamble to the first real
            # phase: dummy matmuls keep the HAM activity window busy; their
            # PSUM bank is reused later with start=True which clears it
            warm = wpool.tile([128, 32 + WARM_N], BF16, tag="warm")
            nc.vector.memset(warm[:], 0.0)
            pwarm = ppool.tile([128, NCH], F32, tag="ps", name="ps_warm")
            for wi in range(WARM_CNT):
                nc.tensor.matmul(pwarm[0:32, 0:WARM_N], warm[:, 0:32],
                                 warm[:, 32:32 + WARM_N],
                                 start=(wi == 0), stop=(wi == WARM_CNT - 1),
                                 tile_position=(0, 0))

            def xplane(p):
                for si, (p0, p1) in enumerate(XSLABS):
                    if p < p1:
                        return XPG[si], p - p0
                raise AssertionError

            g0 = 0
            nb0 = 0  # running bank counter (output bank_seq index)
            for gi, gsz in enumerate(GROUP_SIZES):
                gch = CHUNKS[g0 : g0 + gsz]
                nbank = len(gch) // 4
                ptiles = [ppool.tile([128, NCH], F32, tag="ps", name=f"ps_{gi}_{bi}")
                          for bi in range(nbank)]
                for t, (kind, dx, dy, dz) in enumerate(ctiles):
                    if kind == "full":
                        lhsT = WF[:, dx * 9 + dy * 3 + dz, :]
                    else:
                        lhsT = WT[:, dx, :]  # dx is the tail pass index here
                        da, db, dc = TAIL_BASES[dx]
                    for bi in range(nbank):
                        P = ptiles[bi]
                        for j in range(4):
                            ox, h = gch[bi * 4 + j]
                            y0 = 15 * h
                            if kind == "full":
                                xt, lp = xplane(ox + dx)
                                rhs = xt[:, lp, y0 + dy : y0 + dy + 15, dz : dz + 30]
                            else:
                                rhs = T[:, ox + da, y0 + db : y0 + db + 15,
                                        dc : dc + 30]
                            nc.tensor.matmul(
                                P[32 * j : 32 * (j + 1), :],
                                lhsT,
                                rhs,
                                start=(t == 0),
                                stop=(t == last),
                                tile_position=(0, 32 * j),
                            )
                st = spool.tile([128, nbank * NCH], BF16, tag="st",
                                padded_shape=[128, 4 * NCH], name=f"st_{gi}")
                for bi in range(nbank):
                    nc.vector.tensor_copy(st[:, bi * NCH : (bi + 1) * NCH],
                                          ptiles[bi][:])
                nc.sync.dma_start(out_d[:, nb0 : nb0 + nbank, :], st[:])
                g0 += gsz
                nb0 += nbank

    nc.compile()
    return nc


def _get_nc():
    if "nc" not in _CACHE:
        _CACHE["nc"] = build_nc()
    return _CACHE["nc"]


def _prep_inputs(x, W):
    bf16 = ml_dtypes.bfloat16
    xr = np.ascontiguousarray(x.reshape(8, CIN, DIM, DIM, DIM)).astype(bf16)
    Wr = W.reshape(COUT, CIN, 3, 3, 3).astype(np.float32)

    # host-built shifted tail: block j holds the 16 tail channels shifted
    # flat by TAIL_SHIFTS[j] (zero-fill past the end)
    tails = xr[:, 128:144].reshape(8, 16, NFLAT)
    xt = np.zeros((8, T_ROWS, NFLAT), bf16)
    for j, s in enumerate(TAIL_SHIFTS):
        xt[:, 16 * j : 16 * j + 16, 0 : NFLAT - s] = tails[:, :, s:]
    xt = xt.reshape(8, T_ROWS, DIM, DIM, DIM)

    wf = np.ascontiguousarray(
        Wr[:, :128].reshape(COUT, 128, 27).transpose(1, 2, 0)
    ).astype(bf16)

    # tail weights: pass k reads the tile at AP base TAIL_BASES[k]; block j
    # contributes tap base+shift_j when that decodes to a valid (A,B,C)
    wt = np.zeros((T_ROWS, N_TAIL, COUT), np.float32)
    tailW = Wr[:, 128:144]  # [co, t, A, B, C]
    used = set()
    for k, (da, db, dc) in enumerate(TAIL_BASES):
        bf = 1024 * da + 32 * db + dc
        for j, s in enumerate(TAIL_SHIFTS):
            g = bf + s
            A, rem = divmod(g, 1024)
            B, C = divmod(rem, 32)
            if A <= 2 and B <= 2 and C <= 2 and g not in used:
                used.add(g)
                wt[16 * j : 16 * j + 16, k] = tailW[:, :, A, B, C].T
    assert len(used) == 27, len(used)
    wt = wt.astype(bf16)

    return [{"x": np.ascontiguousarray(xr[b, :128]), "xt": xt[b], "wf": wf, "wt": wt}
            for b in range(N_CORES)]


def kernel(x, W, _trace=False):
    nc = _get_nc()
    in_maps = _prep_inputs(np.asarray(x), np.asarray(W))
    res = None
    for attempt in range(3):
        try:
            res = run_bass_kernel_spmd(nc, in_maps, list(range(N_CORES)), trace=_trace)
            break
        except Exception:
            # rare transient NRT_EXEC_UNIT_UNRECOVERABLE flakes; retry
            if attempt == 2:
                raise
            import time as _time
            _time.sleep(2.0)
    full = np.empty((N_CORES, COUT, ODIM, ODIM, ODIM), np.float32)
    for b in range(N_CORES):
        o = np.asarray(res.results[b]["out"]).astype(np.float32)
        nb = 0
        g0 = 0
        for gsz in GROUP_SIZES:
            for bi in range(gsz // 4):
                for j in range(4):
                    ox, h = CHUNKS[g0 + 4 * bi + j]
                    full[b, :, ox, 15 * h : 15 * h + 15, :] = (
                        o[32 * j : 32 * j + 32, nb].reshape(COUT, 15, 30))
                nb += 1
            g0 += gsz
    if _trace:
        return full, res
    return full


# revision 6
# speedup vs baseline: 1.0061x; 1.0061x over previous
"""Batched DWI 3D conv as implicit GEMM on 8 TRN2 NeuronCores.

Problem: x [8, 12, 12, 32, 32, 32] f32, W [32, 12, 12, 3, 3, 3] f32
         -> out [8, 32, 30, 30, 30] f32  (VALID 3D conv, c_in = 144)

Strategy (data-parallel over batch, one batch element per core):
  - x viewed as [144, 32, 32, 32] bf16 in SBUF; a kernel offset (dx, dy, dz)
    is a pure free-dim shift, so the conv is a chain of shifted matmuls
    accumulated in PSUM: out[co, n] += W_d^T @ x[:, n + shift(d)].
  - c_out = 32 fills only 1/4 of the PE array columns, so 4 col-tiled
    matmuls run concurrently (tile_position=(0, 32j)), each computing a
    different output chunk into its own 32-partition PSUM slice. Measured:
    a 4-matmul phase streams in ~190 ns = one N=450 pass at the warm
    2.4 GHz PE clock, LDWEIGHTS hidden.
  - c_in = 144 = 128 + 16. The 128-channel body: 27 passes (one per tap),
    K=128, shifts via AP offsets (plane index = dx, row = dy, col = dz).
  - The 16-channel tail is packed as a host-built [128, .] tile of 8
    shifted copies (shifts SHIFTS below). 5 tail passes with AP base
    offsets BASES cover all 27 taps exactly once (verified cover; the
    13 surplus (pass, block) slots carry zero weights). 32 passes per
    chunk total vs the naive 27 + 6.
  - Per output chunk (fixed ox, 15 y-rows, 30 z): 32 matmul phases
    accumulate into one PSUM-bank slice, N = 450.
  - DMA pieces are issued in deadline order (each piece lands just before
    the phase that first consumes it); issue cost on the sync queue is
    ~0.65 us per DMA_DIRECT2D, so the first x piece is small to get the
    PE started early. Outputs are stored as bf16 (cast back on host) to
    halve store traffic. Dummy matmuls bridge the PE from the framework
    preamble to the first real phase so the HAM clock gate (4/8 -> 8/8
    after ~3.4 us of sustained activity) is warming during the load.
  - All DMAs stay on the sync HWDGE ring: moving any traffic to the ACT
    HWDGE ring or gpsimd SWDGE measurably degrades aggregate bandwidth.
  Stall budget (from perfetto): ~6.8 us framework preamble, first real
  matmul ~10 us, 480 phases x ~190 ns, one known-unexplained ~4 us
  tensor-engine freeze mid-run (firmware/power; survives scheduling).
"""

import numpy as np
import ml_dtypes

import concourse.bass as bass
import concourse.bacc as bacc
import concourse.mybir as mybir
import concourse.tile as tile
from concourse.bass_utils import run_bass_kernel_spmd

BF16 = mybir.dt.bfloat16
F32 = mybir.dt.float32

N_CORES = 8
CIN = 144
COUT = 32
DIM = 32
ODIM = 30
NCH = 450  # one chunk = 15 y-rows x 30 z
NFLAT = DIM * DIM * DIM
CHUNKS = [(ox, h) for ox in range(ODIM) for h in (0, 1)]  # 60 chunks
# chunks per group (each group = nbank psum banks x 4 col tiles); group sizes
# ramp up so the PE starts after only the first small x slab lands
GROUP_SIZES = [4, 8, 16, 16, 12, 4]
# x body loaded in plane slabs sized to stay ahead of PE consumption
XSLABS = [(0, 2), (2, 4), (4, 8), (8, 16), (16, 24), (24, 32)]
# tail: 8 pre-shifted copies of the 16 tail channels; 5 passes with AP base
# offsets cover all 27 taps exactly once (host asserts the cover)
T_ROWS = 128
TAIL_SHIFTS = [0, 32, 64, 1024, 1057, 1058, 2048, 2112]
TAIL_BASES = [(0, 0, 0), (0, 0, 1), (0, 0, 2), (0, 1, 0), (1, 0, 0)]
TPIECES = [(0, 3), (3, 8), (8, 16), (16, 32)]
N_TAIL = len(TAIL_BASES)
WARM_N = 128
WARM_CNT = 32

_CACHE = {}


def _ctiles():
    out = []
    for dx in range(3):
        for dy in range(3):
            for dz in range(3):
                out.append(("full", dx, dy, dz))
    for k, (da, db, dc) in enumerate(TAIL_BASES):
        out.append(("tail", k, 0, 0))
    return out


def build_nc():
    nc = bacc.Bacc(None, target_bir_lowering=False)
    xin = nc.dram_tensor("x", [128, DIM, DIM, DIM], BF16, kind="ExternalInput")
    xt_d = nc.dram_tensor("xt", [T_ROWS, DIM, DIM, DIM], BF16, kind="ExternalInput")
    wf_d = nc.dram_tensor("wf", [128, 27, COUT], BF16, kind="ExternalInput")
    wt_d = nc.dram_tensor("wt", [T_ROWS, N_TAIL, COUT], BF16, kind="ExternalInput")
    # output laid out [partition = 32*colgroup + co, bank_seq, 450] so each
    # group's store is one DMA with contiguous per-partition records; bf16
    # to halve store traffic (host casts back to f32)
    n_banks_total = sum(g // 4 for g in GROUP_SIZES)
    out_d = nc.dram_tensor("out", [128, n_banks_total, NCH], BF16, kind="ExternalOutput")

    ctiles = _ctiles()
    last = len(ctiles) - 1

    with tile.TileContext(nc) as tc:
        with (
            tc.tile_pool(name="wpool", bufs=1) as wpool,
            tc.tile_pool(name="xpool", bufs=1) as xpool,
            tc.tile_pool(name="tpool", bufs=1) as tpool,
            tc.tile_pool(name="spool", bufs=3) as spool,
            tc.tile_pool(name="ppool", bufs=8, space="PSUM") as ppool,
        ):
            WF = wpool.tile([128, 27, COUT], BF16, tag="wf")
            WT = wpool.tile([T_ROWS, N_TAIL, COUT], BF16, tag="wt")

            XPG = []
            for si, (p0, p1) in enumerate(XSLABS):
                t = xpool.tile([128, p1 - p0, DIM, DIM], BF16, tag=f"xp{si}")
                XPG.append(t)
            T = tpool.tile([T_ROWS, DIM, DIM, DIM], BF16, tag="tail")

            def load_slab(si):
                p0, p1 = XSLABS[si]
                nc.sync.dma_start(XPG[si][:], xin[:, p0:p1, :, :])

            def load_tailp(pi):
                a, b = TPIECES[pi]
                nc.sync.dma_start(T[:, a:b, :, :], xt_d[:, a:b, :, :])

            # issue order = deadline order: each piece lands just ahead of
            # the phase that first consumes it; the first pieces are tiny
            # (ctile-0 weight column, 2 x planes) so the PE starts early
            load_slab(0)                       # planes 0-1
            nc.sync.dma_start(WF[:, 0:1, :], wf_d[:, 0:1, :])  # ctile-0 col
            nc.sync.dma_start(WF[:, 1:27, :], wf_d[:, 1:27, :])
            load_slab(1)                       # planes 2-3
            load_tailp(0)                      # tail planes 0-2
            load_slab(2)                       # planes 4-7
            nc.sync.dma_start(WT[:], wt_d[:])  # tail weights
            load_tailp(1)                      # tail planes 3-7
            load_slab(3)                       # planes 8-15
            load_tailp(2)                      # tail planes 8-15
            load_slab(4)                       # planes 16-23
            load_slab(5)                       # planes 24-31
            load_tailp(3)                      # tail planes 16-31

            # bridge the PE from the framework preamble to the first real
            # phase: dummy matmuls keep the HAM activity window busy; their
            # PSUM bank is reused later with start=True which clears it
            warm = wpool.tile([128, 32 + WARM_N], BF16, tag="warm")
            nc.vector.memset(warm[:], 0.0)
            pwarm = ppool.tile([128, NCH], F32, tag="ps", name="ps_warm")
            for wi in range(WARM_CNT):
                nc.tensor.matmul(pwarm[0:32, 0:WARM_N], warm[:, 0:32],
                                 warm[:, 32:32 + WARM_N],
                                 start=(wi == 0), stop=(wi == WARM_CNT - 1),
                                 tile_position=(0, 0))

            def xplane(p):
                for si, (p0, p1) in enumerate(XSLABS):
                    if p < p1:
                        return XPG[si], p - p0
                raise AssertionError

            g0 = 0
            nb0 = 0  # running bank counter (output bank_seq index)
            for gi, gsz in enumerate(GROUP_SIZES):
                gch = CHUNKS[g0 : g0 + gsz]
                nbank = len(gch) // 4
                ptiles = [ppool.tile([128, NCH], F32, tag="ps", name=f"ps_{gi}_{bi}")
                          for bi in range(nbank)]
                for t, (kind, dx, dy, dz) in enumerate(ctiles):
                    if kind == "full":
                        lhsT = WF[:, dx * 9 + dy * 3 + dz, :]
                    else:
                        lhsT = WT[:, dx, :]  # dx is the tail pass index here
                        da, db, dc = TAIL_BASES[dx]
                    for bi in range(nbank):
                        P = ptiles[bi]
                        for j in range(4):
                            ox, h = gch[bi * 4 + j]
                            y0 = 15 * h
                            if kind == "full":
                                xt, lp = xplane(ox + dx)
                                rhs = xt[:, lp, y0 + dy : y0 + dy + 15, dz : dz + 30]
                            else:
                                rhs = T[:, ox + da, y0 + db : y0 + db + 15,
                                        dc : dc + 30]
                            nc.tensor.matmul(
                                P[32 * j : 32 * (j + 1), :],
                                lhsT,
                                rhs,
                                start=(t == 0),
                                stop=(t == last),
                                tile_position=(0, 32 * j),
                            )
                st = spool.tile([128, nbank * NCH], BF16, tag="st",
                                padded_shape=[128, 4 * NCH], name=f"st_{gi}")
                for bi in range(nbank):
                    nc.vector.tensor_copy(st[:, bi * NCH : (bi + 1) * NCH],
                                          ptiles[bi][:])
                nc.sync.dma_start(out_d[:, nb0 : nb0 + nbank, :], st[:])
                g0 += gsz
                nb0 += nbank

    nc.compile()
    return nc


def _get_nc():
    if "nc" not in _CACHE:
        _CACHE["nc"] = build_nc()
    return _CACHE["nc"]


def _prep_inputs(x, W):
    bf16 = ml_dtypes.bfloat16
    xr = np.ascontiguousarray(x.reshape(8, CIN, DIM, DIM, DIM)).astype(bf16)
    Wr = W.reshape(COUT, CIN, 3, 3, 3).astype(np.float32)

    # host-built shifted tail: block j holds the 16 tail channels shifted
    # flat by TAIL_SHIFTS[j] (zero-fill past the end)
    tails = xr[:, 128:144].reshape(8, 16, NFLAT)
    xt = np.zeros((8, T_ROWS, NFLAT), bf16)
    for j, s in enumerate(TAIL_SHIFTS):
        xt[:, 16 * j : 16 * j + 16, 0 : NFLAT - s] = tails[:, :, s:]
    xt = xt.reshape(8, T_ROWS, DIM, DIM, DIM)

    wf = np.ascontiguousarray(
        Wr[:, :128].reshape(COUT, 128, 27).transpose(1, 2, 0)
    ).astype(bf16)

    # tail weights: pass k reads the tile at AP base TAIL_BASES[k]; block j
    # contributes tap base+shift_j when that decodes to a valid (A,B,C)
    wt = np.zeros((T_ROWS, N_TAIL, COUT), np.float32)
    tailW = Wr[:, 128:144]  # [co, t, A, B, C]
    used = set()
    for k, (da, db, dc) in enumerate(TAIL_BASES):
        bf = 1024 * da + 32 * db + dc
        for j, s in enumerate(TAIL_SHIFTS):
            g = bf + s
            A, rem = divmod(g, 1024)
            B, C = divmod(rem, 32)
            if A <= 2 and B <= 2 and C <= 2 and g not in used:
                used.add(g)
                wt[16 * j : 16 * j + 16, k] = tailW[:, :, A, B, C].T
    assert len(used) == 27, len(used)
    wt = wt.astype(bf16)

    return [{"x": np.ascontiguousarray(xr[b, :128]), "xt": xt[b], "wf": wf, "wt": wt}
            for b in range(N_CORES)]


def kernel(x, W, _trace=False):
    nc = _get_nc()
    in_maps = _prep_inputs(np.asarray(x), np.asarray(W))
    res = None
    for attempt in range(3):
        try:
            res = run_bass_kernel_spmd(nc, in_maps, list(range(N_CORES)), trace=_trace)
            break
        except Exception:
            # rare transient NRT_EXEC_UNIT_UNRECOVERABLE flakes; retry
            if attempt == 2:
                raise
            import time as _time
            _time.sleep(2.0)
    full = np.empty((N_CORES, COUT, ODIM, ODIM, ODIM), np.float32)
    for b in range(N_CORES):
        o = np.asarray(res.results[b]["out"]).astype(np.float32)
        nb = 0
        g0 = 0
        for gsz in GROUP_SIZES:
            for bi in range(gsz // 4):
                for j in range(4):
                    ox, h = CHUNKS[g0 + 4 * bi + j]
                    full[b, :, ox, 15 * h : 15 * h + 15, :] = (
                        o[32 * j : 32 * j + 32, nb].reshape(COUT, 15, 30))
                nb += 1
            g0 += gsz
    if _trace:
        return full, res
    return full


# revision 8
# speedup vs baseline: 1.0190x; 1.0128x over previous
"""Batched DWI 3D conv as implicit GEMM on 8 TRN2 NeuronCores.

Problem: x [8, 12, 12, 32, 32, 32] f32, W [32, 12, 12, 3, 3, 3] f32
         -> out [8, 32, 30, 30, 30] f32  (VALID 3D conv, c_in = 144)

Strategy (data-parallel over batch, one batch element per core):
  - x viewed as [144, 32, 32, 32] bf16 in SBUF; a kernel offset (dx, dy, dz)
    is a pure free-dim shift, so the conv is a chain of shifted matmuls
    accumulated in PSUM: out[co, n] += W_d^T @ x[:, n + shift(d)].
  - c_out = 32 fills only 1/4 of the PE array columns, so 4 col-tiled
    matmuls run concurrently (tile_position=(0, 32j)), each computing a
    different output chunk into its own 32-partition PSUM slice. Measured:
    a 4-matmul phase streams in ~190 ns = one N=450 pass at the warm
    2.4 GHz PE clock, LDWEIGHTS hidden.
  - c_in = 144 = 128 + 16. The 128-channel body: 27 passes (one per tap),
    K=128, shifts via AP offsets (plane index = dx, row = dy, col = dz).
  - The 16-channel tail is packed as a host-built [128, .] tile of 8
    shifted copies (shifts SHIFTS below). 5 tail passes with AP base
    offsets BASES cover all 27 taps exactly once (verified cover; the
    13 surplus (pass, block) slots carry zero weights). 32 passes per
    chunk total vs the naive 27 + 6.
  - Per output chunk (fixed ox, 15 y-rows, 30 z): 32 matmul phases
    accumulate into one PSUM-bank slice, N = 450.
  - DMA pieces are issued in deadline order (each piece lands just before
    the phase that first consumes it); issue cost on the sync queue is
    ~0.65 us per DMA_DIRECT2D, so the first x piece is small to get the
    PE started early. Outputs are stored as bf16 (cast back on host) to
    halve store traffic. Dummy matmuls bridge the PE from the framework
    preamble to the first real phase so the HAM clock gate (4/8 -> 8/8
    after ~3.4 us of sustained activity) is warming during the load.
  - All DMAs stay on the sync HWDGE ring: moving any traffic to the ACT
    HWDGE ring or gpsimd SWDGE measurably degrades aggregate bandwidth.
  Stall budget (from perfetto): ~6.8 us framework preamble, first real
  matmul ~10 us, 480 phases x ~190 ns, one known-unexplained ~4 us
  tensor-engine freeze mid-run (firmware/power; survives scheduling).
"""

import numpy as np
import ml_dtypes

import concourse.bass as bass
import concourse.bacc as bacc
import concourse.mybir as mybir
import concourse.tile as tile
from concourse.bass_utils import run_bass_kernel_spmd

BF16 = mybir.dt.bfloat16
F32 = mybir.dt.float32

N_CORES = 8
CIN = 144
COUT = 32
DIM = 32
ODIM = 30
NCH = 450  # one chunk = 15 y-rows x 30 z
NFLAT = DIM * DIM * DIM
CHUNKS = [(ox, h) for ox in range(ODIM) for h in (0, 1)]  # 60 chunks
# chunks per group (each group = nbank psum banks x 4 col tiles); group sizes
# ramp up so the PE starts after only the first small x slab lands
GROUP_SIZES = [4, 4, 8, 16, 16, 8, 4]
# x body loaded in plane slabs sized to stay ahead of PE consumption
XSLABS = [(0, 2), (2, 4), (4, 8), (8, 16), (16, 24), (24, 32)]
# tail: 8 pre-shifted copies of the 16 tail channels; 5 passes with AP base
# offsets cover all 27 taps exactly once (host asserts the cover)
T_ROWS = 128
TAIL_SHIFTS = [0, 32, 64, 1024, 1057, 1058, 2048, 2112]
TAIL_BASES = [(0, 0, 0), (0, 0, 1), (0, 0, 2), (0, 1, 0), (1, 0, 0)]
TPIECES = [(0, 3), (3, 8), (8, 16), (16, 32)]
N_TAIL = len(TAIL_BASES)
WARM_N = 128
WARM_CNT = 32

_CACHE = {}


def _ctiles():
    out = []
    for dx in range(3):
        for dy in range(3):
            for dz in range(3):
                out.append(("full", dx, dy, dz))
    for k, (da, db, dc) in enumerate(TAIL_BASES):
        out.append(("tail", k, 0, 0))
    return out


def build_nc():
    nc = bacc.Bacc(None, target_bir_lowering=False)
    xin = nc.dram_tensor("x", [128, DIM, DIM, DIM], BF16, kind="ExternalInput")
    xt_d = nc.dram_tensor("xt", [T_ROWS, DIM, DIM, DIM], BF16, kind="ExternalInput")
    wf_d = nc.dram_tensor("wf", [128, 27, COUT], BF16, kind="ExternalInput")
    wt_d = nc.dram_tensor("wt", [T_ROWS, N_TAIL, COUT], BF16, kind="ExternalInput")
    # output laid out [partition = 32*colgroup + co, bank_seq, 450] so each
    # group's store is one DMA with contiguous per-partition records; bf16
    # to halve store traffic (host casts back to f32)
    n_banks_total = sum(g // 4 for g in GROUP_SIZES)
    out_d = nc.dram_tensor("out", [128, n_banks_total, NCH], BF16, kind="ExternalOutput")

    ctiles = _ctiles()
    last = len(ctiles) - 1

    with tile.TileContext(nc) as tc:
        with (
            tc.tile_pool(name="wpool", bufs=1) as wpool,
            tc.tile_pool(name="xpool", bufs=1) as xpool,
            tc.tile_pool(name="tpool", bufs=1) as tpool,
            tc.tile_pool(name="spool", bufs=3) as spool,
            tc.tile_pool(name="ppool", bufs=8, space="PSUM") as ppool,
        ):
            WF = wpool.tile([128, 27, COUT], BF16, tag="wf")
            WT = wpool.tile([T_ROWS, N_TAIL, COUT], BF16, tag="wt")

            XPG = []
            for si, (p0, p1) in enumerate(XSLABS):
                t = xpool.tile([128, p1 - p0, DIM, DIM], BF16, tag=f"xp{si}")
                XPG.append(t)
            T = tpool.tile([T_ROWS, DIM, DIM, DIM], BF16, tag="tail")

            def load_slab(si):
                p0, p1 = XSLABS[si]
                nc.sync.dma_start(XPG[si][:], xin[:, p0:p1, :, :])

            def load_tailp(pi):
                a, b = TPIECES[pi]
                nc.sync.dma_start(T[:, a:b, :, :], xt_d[:, a:b, :, :])

            # issue order = deadline order: each piece lands just ahead of
            # the phase that first consumes it; the first pieces are tiny
            # (ctile-0 weight column, 2 x planes) so the PE starts early
            load_slab(0)                       # planes 0-1
            nc.sync.dma_start(WF[:, 0:1, :], wf_d[:, 0:1, :])  # ctile-0 col
            nc.sync.dma_start(WT[:], wt_d[:])  # tail weights (tiny)
            nc.sync.dma_start(WF[:, 1:27, :], wf_d[:, 1:27, :])
            load_slab(1)                       # planes 2-3
            load_tailp(0)                      # tail planes 0-2
            load_slab(2)                       # planes 4-7
            load_tailp(1)                      # tail planes 3-7
            load_slab(3)                       # planes 8-15
            load_tailp(2)                      # tail planes 8-15
            load_slab(4)                       # planes 16-23
            load_slab(5)                       # planes 24-31
            load_tailp(3)                      # tail planes 16-31

            # bridge the PE from the framework preamble to the first real
            # phase: dummy matmuls keep the HAM activity window busy; their
            # PSUM bank is reused later with start=True which clears it
            warm = wpool.tile([128, 32 + WARM_N], BF16, tag="warm")
            nc.vector.memset(warm[:], 0.0)
            pwarm = ppool.tile([128, NCH], F32, tag="ps", name="ps_warm")
            for wi in range(WARM_CNT):
                nc.tensor.matmul(pwarm[0:32, 0:WARM_N], warm[:, 0:32],
                                 warm[:, 32:32 + WARM_N],
                                 start=(wi == 0), stop=(wi == WARM_CNT - 1),
                                 tile_position=(0, 0))

            def xplane(p):
                for si, (p0, p1) in enumerate(XSLABS):
                    if p < p1:
                        return XPG[si], p - p0
                raise AssertionError

            g0 = 0
            nb0 = 0  # running bank counter (output bank_seq index)
            for gi, gsz in enumerate(GROUP_SIZES):
                gch = CHUNKS[g0 : g0 + gsz]
                nbank = len(gch) // 4
                ptiles = [ppool.tile([128, NCH], F32, tag="ps", name=f"ps_{gi}_{bi}")
                          for bi in range(nbank)]
                for t, (kind, dx, dy, dz) in enumerate(ctiles):
                    if kind == "full":
                        lhsT = WF[:, dx * 9 + dy * 3 + dz, :]
                    else:
                        lhsT = WT[:, dx, :]  # dx is the tail pass index here
                        da, db, dc = TAIL_BASES[dx]
                    for bi in range(nbank):
                        P = ptiles[bi]
                        for j in range(4):
                            ox, h = gch[bi * 4 + j]
                            y0 = 15 * h
                            if kind == "full":
                                xt, lp = xplane(ox + dx)
                                rhs = xt[:, lp, y0 + dy : y0 + dy + 15, dz : dz + 30]
                            else:
                                rhs = T[:, ox + da, y0 + db : y0 + db + 15,
                                        dc : dc + 30]
                            nc.tensor.matmul(
                                P[32 * j : 32 * (j + 1), :],
                                lhsT,
                                rhs,
                                start=(t == 0),
                                stop=(t == last),
                                tile_position=(0, 32 * j),
                            )
                st = spool.tile([128, nbank * NCH], BF16, tag="st",
                                padded_shape=[128, 4 * NCH], name=f"st_{gi}")
                for bi in range(nbank):
                    nc.vector.tensor_copy(st[:, bi * NCH : (bi + 1) * NCH],
                                          ptiles[bi][:])
                nc.sync.dma_start(out_d[:, nb0 : nb0 + nbank, :], st[:])
                g0 += gsz
                nb0 += nbank

    nc.compile()
    return nc


def _get_nc():
    if "nc" not in _CACHE:
        _CACHE["nc"] = build_nc()
    return _CACHE["nc"]


def _prep_inputs(x, W):
    bf16 = ml_dtypes.bfloat16
    xr = np.ascontiguousarray(x.reshape(8, CIN, DIM, DIM, DIM)).astype(bf16)
    Wr = W.reshape(COUT, CIN, 3, 3, 3).astype(np.float32)

    # host-built shifted tail: block j holds the 16 tail channels shifted
    # flat by TAIL_SHIFTS[j] (zero-fill past the end)
    tails = xr[:, 128:144].reshape(8, 16, NFLAT)
    xt = np.zeros((8, T_ROWS, NFLAT), bf16)
    for j, s in enumerate(TAIL_SHIFTS):
        xt[:, 16 * j : 16 * j + 16, 0 : NFLAT - s] = tails[:, :, s:]
    xt = xt.reshape(8, T_ROWS, DIM, DIM, DIM)

    wf = np.ascontiguousarray(
        Wr[:, :128].reshape(COUT, 128, 27).transpose(1, 2, 0)
    ).astype(bf16)

    # tail weights: pass k reads the tile at AP base TAIL_BASES[k]; block j
    # contributes tap base+shift_j when that decodes to a valid (A,B,C)
    wt = np.zeros((T_ROWS, N_TAIL, COUT), np.float32)
    tailW = Wr[:, 128:144]  # [co, t, A, B, C]
    used = set()
    for k, (da, db, dc) in enumerate(TAIL_BASES):
        bf = 1024 * da + 32 * db + dc
        for j, s in enumerate(TAIL_SHIFTS):
            g = bf + s
            A, rem = divmod(g, 1024)
            B, C = divmod(rem, 32)
            if A <= 2 and B <= 2 and C <= 2 and g not in used:
                used.add(g)
                wt[16 * j : 16 * j + 16, k] = tailW[:, :, A, B, C].T
    assert len(used) == 27, len(used)
    wt = wt.astype(bf16)

    return [{"x": np.ascontiguousarray(xr[b, :128]), "xt": xt[b], "wf": wf, "wt": wt}
            for b in range(N_CORES)]


def kernel(x, W, _trace=False):
    nc = _get_nc()
    in_maps = _prep_inputs(np.asarray(x), np.asarray(W))
    res = None
    for attempt in range(3):
        try:
            res = run_bass_kernel_spmd(nc, in_maps, list(range(N_CORES)), trace=_trace)
            break
        except Exception:
            # rare transient NRT_EXEC_UNIT_UNRECOVERABLE flakes; retry
            if attempt == 2:
                raise
            import time as _time
            _time.sleep(2.0)
    full = np.empty((N_CORES, COUT, ODIM, ODIM, ODIM), np.float32)
    for b in range(N_CORES):
        o = np.asarray(res.results[b]["out"]).astype(np.float32)
        nb = 0
        g0 = 0
        for gsz in GROUP_SIZES:
            for bi in range(gsz // 4):
                for j in range(4):
                    ox, h = CHUNKS[g0 + 4 * bi + j]
                    full[b, :, ox, 15 * h : 15 * h + 15, :] = (
                        o[32 * j : 32 * j + 32, nb].reshape(COUT, 15, 30))
                nb += 1
            g0 += gsz
    if _trace:
        return full, res
    return full
